# revision 3
# baseline (speedup 1.0000x reference)
"""TRN2 Bass/Tile kernel v2: BatchNorm1d + 4-head self-attention + out-proj.

Reference computation (b=4, c=256, n=4096, heads=4, d=64):
    xn   = BN(x)  (training-mode stats over batch+length)
    qkv  = w_qkv @ xn ;  q,k,v  (q scaled by d^-0.5, folded into host wq)
    out  = softmax(q^T k) @ v^T  per (batch, head)
    y    = w_out @ out + b_out

Sharding over 8 NeuronCores: core i handles (batch i//2, query-half i%2).
Exact BN via [256,2] AllReduce of per-core partial stats.

Attention in transposed-score layout S^T[key, query]; per (rb=head-pair, j):
  - scores: row-packed concurrent K=64 bf16 matmul pairs via tile_position:
    head 2rb in PE rows 0-63, head 2rb+1 in rows 64-127 (2 chunks / ~280ns)
  - exp(s-1) into fp8(e4m3): alternating ACT (exact, Exp activation) and
    DVE (Schraudolph bit-trick via tensor_scalar -> uint8) straight from PSUM
  - AV: fp8 DoubleRow matmuls, 256 keys per instruction, with a ones column
    at dim 64 producing the softmax denominator for free
  - normalize on DVE (recip + broadcast-mult), out-projection in bf16
"""

import numpy as np
import ml_dtypes

import concourse.bacc as bacc
import concourse.tile as tile
from concourse import mybir
from concourse.bass_utils import run_bass_kernel_spmd

B, C, N = 4, 256, 4096
H, D = 4, 64
P = 128
CT = C // P            # 2 channel tiles of 128
RB = 2                 # head pairs: rb covers heads (2rb, 2rb+1)
NH = N // 2            # 2048 queries per core
QS = 512               # query block
NQS = NH // QS         # 4
KC = 128               # key chunk
NKC = N // KC          # 32
SLOTS = 2 * NKC        # 64 score slots per (rb, j): slot = 2*c + h2
G = 2                  # slots per exp instruction (1 key-chunk, both heads)
NGR = SLOTS // G       # 32
VP = 80                # padded fp8 AV stationary cols (64 v + 1 ones + 15 zero)
EPS = 1e-5
SCALE = D ** -0.5
NCORES = 8
DEBUG = False

LOG2E = 1.4426950408889634
MARGIN = 1.0           # compute exp(s - MARGIN); cancels in softmax
FE_A = 8.0 * LOG2E
FE_C = -0.44           # calibration for DVE uint8 convert (mean-centering)
FE_B = 56.0 + FE_C - FE_A * MARGIN

F32 = mybir.dt.float32
F32R = mybir.dt.float32r
BF16 = mybir.dt.bfloat16
U8 = mybir.dt.uint8
E4 = mybir.dt.float8e4

# exp groups assigned to ACT (scalar engine); rest go to DVE (which also
# runs the normalize). 18/14 split balances measured per-group costs.
ACT_GROUPS = frozenset(g for g in range(32) if g % 16 not in (1, 3, 5, 7, 9, 11, 14))


def _body(tc, x_mine, x_other, w_qkvT, w_outT, bn_w, bn_b, b_out, out,
          dbg=None):
    from contextlib import ExitStack

    nc = tc.nc
    AF = mybir.ActivationFunctionType
    OP = mybir.AluOpType

    with ExitStack() as ctx:
        big = ctx.enter_context(tc.tile_pool(name="big", bufs=1))
        small = ctx.enter_context(tc.tile_pool(name="small", bufs=1))
        epool = ctx.enter_context(tc.tile_pool(name="epool", bufs=2))
        rpool = ctx.enter_context(tc.tile_pool(name="rpool", bufs=2))
        opool = ctx.enter_context(tc.tile_pool(name="opool", bufs=2))
        spool = ctx.enter_context(tc.tile_pool(name="spool", bufs=3, space="PSUM"))
        mmpool = ctx.enter_context(tc.tile_pool(name="mmpool", bufs=2, space="PSUM"))
        dram = ctx.enter_context(tc.tile_pool(name="dram", bufs=1, space="DRAM"))

        # ---- loads ------------------------------------------------------
        x_sb = big.tile([P, CT, N], F32R)  # keys ordered [mine | other]
        xm_r = x_mine.rearrange("(ct p) n -> p ct n", p=P)
        for ct in range(CT):
            for q8 in range(4):
                nc.sync.dma_start(
                    out=x_sb[:, ct, q8 * (NH // 4): (q8 + 1) * (NH // 4)],
                    in_=xm_r[:, ct, q8 * (NH // 4): (q8 + 1) * (NH // 4)],
                )
        nc.sync.dma_start(
            out=x_sb[:, :, NH:N], in_=x_other.rearrange("(ct p) n -> p ct n", p=P)
        )
        wq_sb = big.tile([P, CT, 3 * C], BF16)
        nc.sync.dma_start(out=wq_sb, in_=w_qkvT.rearrange("(ct p) o -> p ct o", p=P))
        wo_sb = big.tile([P, CT, C], BF16)
        nc.sync.dma_start(out=wo_sb, in_=w_outT.rearrange("(ct p) o -> p ct o", p=P))
        bnw_sb = small.tile([P, CT, 1], F32)
        nc.sync.dma_start(out=bnw_sb, in_=bn_w)
        bnb_sb = small.tile([P, CT, 1], F32)
        nc.sync.dma_start(out=bnb_sb, in_=bn_b)
        bo_sb = small.tile([P, CT, 1], F32)
        nc.sync.dma_start(out=bo_sb, in_=b_out)

        # preload the exp activation table while DMAs run
        scr = small.tile([1, 2], F32)
        nc.vector.memset(scr[:, 0:1], 0.0)
        nmarg = small.tile([P, 1], F32)
        nc.vector.memset(nmarg, -MARGIN)
        nc.scalar.activation(out=scr[:, 1:2], in_=scr[:, 0:1], func=AF.Exp,
                             bias=nmarg[0:1])

        # ---- BN stats over my (batch, half) slice + AllReduce -----------
        SG = NH // 512
        stat6 = small.tile([P, CT, SG, 6], F32)
        for ct in range(CT):
            xm = x_sb.bitcast(F32)[:, ct, 0:NH].rearrange("p (s f) -> p s f", f=512)
            for s in range(SG):
                nc.vector.bn_stats(out=stat6[:, ct, s, :], in_=xm[:, s, :])
        mv = small.tile([P, CT, 2], F32)
        for ct in range(CT):
            nc.vector.bn_aggr(out=mv[:, ct, :], in_=stat6[:, ct])

        # pack [mean, E[x^2]] = [mean, var + mean^2]
        ccin_sb = small.tile([P, CT, 2], F32)
        nc.vector.tensor_copy(out=ccin_sb[:, :, 0:1], in_=mv[:, :, 0:1])
        msq = small.tile([P, CT, 1], F32)
        nc.vector.tensor_mul(out=msq, in0=mv[:, :, 0:1], in1=mv[:, :, 0:1])
        nc.vector.tensor_add(out=ccin_sb[:, :, 1:2], in0=mv[:, :, 1:2], in1=msq)

        cc_in = dram.tile([C, 2], F32)
        cc_out = dram.tile([NCORES, C, 2], F32, addr_space="Shared")
        nc.sync.dma_start(out=cc_in.rearrange("(ct p) s -> p ct s", p=P), in_=ccin_sb)
        nc.gpsimd.collective_compute(
            "AllGather",
            OP.bypass,
            replica_groups=[list(range(NCORES))],
            ins=[cc_in.opt()],
            outs=[cc_out.opt()],
        )
        g8 = small.tile([P, NCORES, CT, 2], F32)
        nc.sync.dma_start(
            out=g8, in_=cc_out.rearrange("r (ct p) s -> p r ct s", p=P))
        g8f = g8.rearrange("p r ct s -> p (r ct s)")
        g16 = small.tile([P, 16], F32)
        nc.vector.tensor_add(out=g16, in0=g8f[:, 0:16], in1=g8f[:, 16:32])
        g4t = small.tile([P, 8], F32)
        nc.vector.tensor_add(out=g4t, in0=g16[:, 0:8], in1=g16[:, 8:16])
        gstat = small.tile([P, CT, 2], F32)
        nc.vector.tensor_add(out=gstat.rearrange("p ct s -> p (ct s)"),
                             in0=g4t[:, 0:4], in1=g4t[:, 4:8])

        # zero/ones padding of the fp8 AV stationary tiles (DVE; gpsimd must
        # stay free so the collective isn't queued behind memsets)
        vT_pad = big.tile([P, H, NKC // 2, 2, VP], U8)
        nc.vector.memset(vT_pad[:, :, :, :, D:VP], 0)
        nc.vector.memset(vT_pad[:, :, :, :, D:D + 1].bitcast(E4), 1.0)

        # warm the PE a little during the collective wait
        wt = small.tile([1, P], BF16)
        nc.vector.memset(wt, 1.0)
        for _ in range(24):
            ws = mmpool.tile([P, QS], F32, tag="mm", name="warm")
            nc.tensor.matmul(out=ws[0:1, 0:P], lhsT=wt[0:1, 0:1], rhs=wt,
                             start=True, stop=True)

        # global mean/var -> s = bn_w * rstd, t = bn_b - mean * s
        mean_g = small.tile([P, CT, 1], F32)
        nc.vector.tensor_scalar_mul(out=mean_g, in0=gstat[:, :, 0:1],
                                    scalar1=1.0 / NCORES)
        e2 = small.tile([P, CT, 1], F32)
        nc.vector.tensor_scalar_mul(out=e2, in0=gstat[:, :, 1:2],
                                    scalar1=1.0 / NCORES)
        var_g = small.tile([P, CT, 1], F32)
        nc.vector.tensor_mul(out=var_g, in0=mean_g, in1=mean_g)
        nc.vector.tensor_sub(out=var_g, in0=e2, in1=var_g)
        eps_sb = small.tile([P, 1], F32)
        nc.vector.memset(eps_sb, EPS)
        sd = small.tile([P, CT, 1], F32)
        nc.scalar.activation(out=sd, in_=var_g, func=AF.Sqrt, bias=eps_sb)
        rstd = small.tile([P, CT, 1], F32)
        nc.vector.reciprocal(out=rstd, in_=sd)
        s_sb = small.tile([P, CT, 1], F32)
        nc.vector.tensor_mul(out=s_sb, in0=bnw_sb, in1=rstd)
        t_sb = small.tile([P, CT, 1], F32)
        nc.vector.tensor_mul(out=t_sb, in0=mean_g, in1=s_sb)
        nc.vector.tensor_sub(out=t_sb, in0=bnb_sb, in1=t_sb)

        # ---- xn = s*x + t, cast to bf16 (DVE: ct0, ACT: ct1) ------------
        xb = big.tile([P, CT, N], BF16)
        XSL = 1024
        for sl in range(N // XSL):
            nc.vector.tensor_scalar(
                out=xb[:, 0, sl * XSL:(sl + 1) * XSL],
                in0=x_sb.bitcast(F32)[:, 0, sl * XSL:(sl + 1) * XSL],
                scalar1=s_sb[:, 0], scalar2=t_sb[:, 0],
                op0=OP.mult, op1=OP.add,
            )
            nc.scalar.activation(
                out=xb[:, 1, sl * XSL:(sl + 1) * XSL],
                in_=x_sb.bitcast(F32)[:, 1, sl * XSL:(sl + 1) * XSL],
                func=AF.Identity, bias=t_sb[:, 1], scale=s_sb[:, 1],
            )

        # ---- QKV projections (all bf16) ---------------------------------
        # k_nat[0:64, rb, :]   = head 2rb   k-channels, natural key order
        # k_nat[64:128, rb, :] = head 2rb+1
        k_nat = big.tile([P, RB, N], BF16)
        q_pad = big.tile([P, RB * NQS, QS], BF16)  # verbatim q-proj psum copies
        attn_sb = big.tile([P, RB, NH], BF16)
        eb_r = epool  # alias

        # k projection (ACT copies)
        for rb in range(RB):
            for jk in range(N // QS):
                ps = mmpool.tile([P, QS], F32, tag="mm")
                for ct in range(CT):
                    nc.tensor.matmul(
                        out=ps,
                        lhsT=wq_sb[:, ct, C + rb * P: C + (rb + 1) * P],
                        rhs=xb[:, ct, jk * QS:(jk + 1) * QS],
                        start=(ct == 0), stop=(ct == CT - 1),
                    )
                nc.scalar.copy(out=k_nat[:, rb, jk * QS:(jk + 1) * QS], in_=ps)

        def qproj(rb, j):
            ps = mmpool.tile([P, QS], F32, tag="mm", name="qps")
            for ct in range(CT):
                nc.tensor.matmul(
                    out=ps,
                    lhsT=wq_sb[:, ct, rb * P:(rb + 1) * P],
                    rhs=xb[:, ct, j * QS:(j + 1) * QS],
                    start=(ct == 0), stop=(ct == CT - 1),
                )
            nc.vector.tensor_copy(out=q_pad[:, rb * NQS + j, :], in_=ps)

        # q for j=0 first (unblocks attention), then v, then remaining q
        for rb in range(RB):
            qproj(rb, 0)

        # v projection, transposed: psum [key, (h d)] pairs of chunks share
        # one bank; copy+cast straight to fp8 with per-head placement (DVE)
        for pv in range(NKC // 2):
            ps = mmpool.tile([P, 2, C], F32, tag="mm")
            for ko in range(2):
                nb = 2 * pv + ko
                for ct in range(CT):
                    nc.tensor.matmul(
                        out=ps[:, ko, :],
                        lhsT=xb[:, ct, nb * KC:(nb + 1) * KC],
                        rhs=wq_sb[:, ct, 2 * C:3 * C],
                        start=(ct == 0), stop=(ct == CT - 1),
                    )
            for ko in range(2):
                nc.vector.tensor_copy(
                    out=vT_pad[:, :, pv, ko, 0:D].bitcast(E4),
                    in_=ps[:, ko, :].rearrange("p (h c) -> p h c", c=D),
                )

        for j in range(1, NQS):
            for rb in range(RB):
                qproj(rb, j)

        # ---- attention --------------------------------------------------
        out_r = out.rearrange("(rb p) n -> p rb n", p=P)

        def normalize(avp_, rb_, h2_, j_):
            dstage = rpool.tile([1, QS], F32, tag="ds")
            nc.vector.tensor_copy(out=dstage, in_=avp_[D:D + 1, :])
            r_sb = rpool.tile([1, QS], F32, tag="r")
            nc.vector.reciprocal_approx_fast(out=r_sb, in_=dstage)
            rbc = rpool.tile([D, QS], F32, tag="rbc")
            nc.gpsimd.partition_broadcast(rbc, r_sb)
            nc.vector.tensor_tensor(
                out=attn_sb[h2_ * D:(h2_ + 1) * D, rb_, j_ * QS:(j_ + 1) * QS],
                in0=avp_[0:D, :], in1=rbc, op=OP.mult,
            )

        def outproj(j_):
            for rbo in range(RB):
                po = mmpool.tile([P, QS], F32, tag="mm", name="po")
                for ct in range(CT):
                    nc.tensor.matmul(
                        out=po,
                        lhsT=wo_sb[:, ct, rbo * P:(rbo + 1) * P],
                        rhs=attn_sb[:, ct, j_ * QS:(j_ + 1) * QS],
                        start=(ct == 0), stop=(ct == CT - 1),
                    )
                o_t = opool.tile([P, QS], F32, tag="o")
                nc.scalar.activation(out=o_t, in_=po, func=AF.Identity,
                                     bias=bo_sb[:, rbo], scale=1.0)
                nc.sync.dma_start(out=out_r[:, rbo, j_ * QS:(j_ + 1) * QS],
                                  in_=o_t)

        for j in range(NQS):
            for rb in range(RB):
                eb = epool.tile([P, SLOTS, QS], U8, tag="e")
                eb5 = eb.rearrange("pt (p ko h) c -> pt p ko h c", ko=2, h=2)
                avp = [None, None]
                pend = []  # delayed AV emits: (h2, pair, start, stop)

                def flush_pend(upto_g):
                    while pend and pend[0][0] <= upto_g:
                        (_, h2_, p_, st_, sp_) = pend.pop(0)
                        nc.tensor.matmul(
                            out=avp[h2_][0:VP, :],
                            lhsT=vT_pad[:, 2 * rb + h2_, p_].bitcast(E4),
                            rhs=eb5[:, p_, :, h2_, :].bitcast(E4),
                            start=st_, stop=sp_,
                            perf_mode=mybir.MatmulPerfMode.DoubleRow,
                        )
                        if sp_:
                            normalize(avp[h2_], rb, h2_, j)

                avp[0] = mmpool.tile([P, QS], F32, tag="mm", name="avp0")
                avp[1] = mmpool.tile([P, QS], F32, tag="mm", name="avp1")
                for g in range(NGR):
                    gs = min(G, SLOTS - g * G)
                    # AVs whose exp finished >= 2 groups ago sit at the PE
                    # queue head without blocking (their input is long done)
                    flush_pend(g - 2)
                    st = spool.tile([P, G, QS], F32, tag="sp")
                    for u in range(gs):
                        slot = g * G + u
                        c, h2 = slot // 2, slot % 2
                        nc.tensor.matmul(
                            out=st[:, u, :],
                            lhsT=k_nat[h2 * D:(h2 + 1) * D, rb,
                                       c * KC:(c + 1) * KC],
                            rhs=q_pad[h2 * D:(h2 + 1) * D, rb * NQS + j, :],
                            start=True, stop=True,
                            tile_position=(h2 * D, 0),
                        )
                    if g in ACT_GROUPS:
                        nc.scalar.activation(
                            out=eb[:, g * G:g * G + gs, :].bitcast(E4),
                            in_=st[:, 0:gs, :], func=AF.Exp,
                            bias=nmarg, scale=1.0,
                        )
                    else:
                        nc.vector.tensor_scalar(
                            out=eb[:, g * G:g * G + gs, :],
                            in0=st[:, 0:gs, :],
                            scalar1=FE_A, scalar2=FE_B,
                            op0=OP.mult, op1=OP.add,
                        )
                    for u in range(gs):
                        slot = g * G + u
                        if slot >= 2 and (slot - 2) % 4 in (0, 1):
                            p_, h2_ = (slot - 2) // 4, (slot - 2) % 4
                            pend.append((g, h2_, p_, p_ == 0,
                                         p_ == NKC // 2 - 1))
                flush_pend(NGR + 2)
                if dbg is not None and j == 0 and rb == 0:
                    nc.sync.dma_start(out=dbg["eb"], in_=eb)
            outproj(j)
        if dbg is not None:
            nc.sync.dma_start(out=dbg["xb"], in_=xb)
            nc.sync.dma_start(out=dbg["q"], in_=q_pad)
            nc.sync.dma_start(out=dbg["k"], in_=k_nat)
            nc.sync.dma_start(out=dbg["attn"], in_=attn_sb)
            nc.sync.dma_start(out=dbg["vt"], in_=vT_pad)


def build():
    nc = bacc.Bacc("TRN2", target_bir_lowering=False, debug=False,
                   num_devices=NCORES)
    x_mine = nc.dram_tensor("x_mine", [C, NH], F32R, kind="ExternalInput").ap()
    x_other = nc.dram_tensor("x_other", [C, NH], F32R, kind="ExternalInput").ap()
    w_qkvT = nc.dram_tensor("w_qkvT", [C, 3 * C], BF16, kind="ExternalInput").ap()
    w_outT = nc.dram_tensor("w_outT", [C, C], BF16, kind="ExternalInput").ap()
    bn_w = nc.dram_tensor("bn_w", [P, CT, 1], F32, kind="ExternalInput").ap()
    bn_b = nc.dram_tensor("bn_b", [P, CT, 1], F32, kind="ExternalInput").ap()
    b_out = nc.dram_tensor("b_out", [P, CT, 1], F32, kind="ExternalInput").ap()
    out = nc.dram_tensor("out", [C, NH], F32, kind="ExternalOutput").ap()
    dbg = None
    if DEBUG:
        dbg = {
            "eb": nc.dram_tensor("dbg_eb", [P, SLOTS, QS], U8, kind="ExternalOutput").ap(),
            "xb": nc.dram_tensor("dbg_xb", [P, CT, N], BF16, kind="ExternalOutput").ap(),
            "q": nc.dram_tensor("dbg_q", [P, RB * NQS, QS], BF16, kind="ExternalOutput").ap(),
            "k": nc.dram_tensor("dbg_k", [P, RB, N], BF16, kind="ExternalOutput").ap(),
            "attn": nc.dram_tensor("dbg_attn", [P, RB, NH], BF16, kind="ExternalOutput").ap(),
            "vt": nc.dram_tensor("dbg_vt", [P, H, NKC // 2, 2, VP], U8, kind="ExternalOutput").ap(),
        }
    with tile.TileContext(nc) as tc:
        _body(tc, x_mine, x_other, w_qkvT, w_outT, bn_w, bn_b, b_out, out,
              dbg)
    nc.compile()
    return nc


_nc_cache = None


def make_in_maps(x, bn_weight, bn_bias, w_qkv, w_out, b_out):
    x = np.ascontiguousarray(np.asarray(x, dtype=np.float32))
    wq = np.asarray(w_qkv, dtype=np.float32).copy()
    wq[0:C, :] *= SCALE  # fold q scaling d^-0.5 into the q rows
    wqT = np.ascontiguousarray(wq.T).astype(ml_dtypes.bfloat16)
    woT = np.ascontiguousarray(np.asarray(w_out, dtype=np.float32).T).astype(
        ml_dtypes.bfloat16)

    def vec_layout(v):
        v = np.asarray(v, dtype=np.float32)
        return np.ascontiguousarray(v.reshape(CT, P).T.reshape(P, CT, 1))

    bnw = vec_layout(bn_weight)
    bnb = vec_layout(bn_bias)
    bo = vec_layout(b_out)
    in_maps = []
    for core in range(NCORES):
        bi, half = divmod(core, 2)
        mine = np.ascontiguousarray(x[bi][:, half * NH:(half + 1) * NH])
        other = np.ascontiguousarray(x[bi][:, (1 - half) * NH:(2 - half) * NH])
        in_maps.append({
            "x_mine": mine, "x_other": other, "w_qkvT": wqT, "w_outT": woT,
            "bn_w": bnw, "bn_b": bnb, "b_out": bo,
        })
    return in_maps


def assemble(results):
    outp = np.empty((B, C, N), np.float32)
    for core in range(NCORES):
        bi, half = divmod(core, 2)
        outp[bi][:, half * NH:(half + 1) * NH] = results[core]["out"]
    return outp


def kernel(x, bn_weight, bn_bias, w_qkv, w_out, b_out):
    global _nc_cache
    if _nc_cache is None:
        _nc_cache = build()
    in_maps = make_in_maps(x, bn_weight, bn_bias, w_qkv, w_out, b_out)
    res = run_bass_kernel_spmd(_nc_cache, in_maps, list(range(NCORES)))
    return assemble(res.results)


# revision 5
# speedup vs baseline: 1.0087x; 1.0087x over previous
"""TRN2 Bass/Tile kernel v2: BatchNorm1d + 4-head self-attention + out-proj.

Reference computation (b=4, c=256, n=4096, heads=4, d=64):
    xn   = BN(x)  (training-mode stats over batch+length)
    qkv  = w_qkv @ xn ;  q,k,v  (q scaled by d^-0.5, folded into host wq)
    out  = softmax(q^T k) @ v^T  per (batch, head)
    y    = w_out @ out + b_out

Sharding over 8 NeuronCores: core i handles (batch i//2, query-half i%2).
Exact BN via [256,2] AllReduce of per-core partial stats.

Attention in transposed-score layout S^T[key, query]; per (rb=head-pair, j):
  - scores: row-packed concurrent K=64 bf16 matmul pairs via tile_position:
    head 2rb in PE rows 0-63, head 2rb+1 in rows 64-127 (2 chunks / ~280ns)
  - exp(s-1) into fp8(e4m3): alternating ACT (exact, Exp activation) and
    DVE (Schraudolph bit-trick via tensor_scalar -> uint8) straight from PSUM
  - AV: fp8 DoubleRow matmuls, 256 keys per instruction, with a ones column
    at dim 64 producing the softmax denominator for free
  - normalize on DVE (recip + broadcast-mult), out-projection in bf16
"""

import numpy as np
import ml_dtypes

import concourse.bacc as bacc
import concourse.tile as tile
from concourse import mybir
from concourse.bass_utils import run_bass_kernel_spmd

B, C, N = 4, 256, 4096
H, D = 4, 64
P = 128
CT = C // P            # 2 channel tiles of 128
RB = 2                 # head pairs: rb covers heads (2rb, 2rb+1)
NH = N // 2            # 2048 queries per core
QS = 512               # query block
NQS = NH // QS         # 4
KC = 128               # key chunk
NKC = N // KC          # 32
SLOTS = 2 * NKC        # 64 score slots per (rb, j): slot = 2*c + h2
G = 2                  # slots per exp instruction (1 key-chunk, both heads)
NGR = SLOTS // G       # 32
VP = 80                # padded fp8 AV stationary cols (64 v + 1 ones + 15 zero)
EPS = 1e-5
SCALE = D ** -0.5
NCORES = 8
DEBUG = False

LOG2E = 1.4426950408889634
MARGIN = 1.0           # compute exp(s - MARGIN); cancels in softmax
FE_A = 8.0 * LOG2E
FE_C = -0.44           # calibration for DVE uint8 convert (mean-centering)
FE_B = 56.0 + FE_C - FE_A * MARGIN

F32 = mybir.dt.float32
F32R = mybir.dt.float32r
BF16 = mybir.dt.bfloat16
U8 = mybir.dt.uint8
E4 = mybir.dt.float8e4

# exp groups assigned to ACT (scalar engine); rest go to DVE (which also
# runs the normalize). 18/14 split balances measured per-group costs.
ACT_GROUPS = frozenset(g for g in range(32) if g % 16 not in (1, 3, 5, 7, 9, 11, 14))


def _body(tc, x_mine, x_other, w_qkvT, w_outT, bn_w, bn_b, b_out, out,
          dbg=None):
    from contextlib import ExitStack

    nc = tc.nc
    AF = mybir.ActivationFunctionType
    OP = mybir.AluOpType

    with ExitStack() as ctx:
        big = ctx.enter_context(tc.tile_pool(name="big", bufs=1))
        small = ctx.enter_context(tc.tile_pool(name="small", bufs=1))
        epool = ctx.enter_context(tc.tile_pool(name="epool", bufs=3))
        rpool = ctx.enter_context(tc.tile_pool(name="rpool", bufs=1))
        opool = ctx.enter_context(tc.tile_pool(name="opool", bufs=1))
        spool = ctx.enter_context(tc.tile_pool(name="spool", bufs=3, space="PSUM"))
        mmpool = ctx.enter_context(tc.tile_pool(name="mmpool", bufs=2, space="PSUM"))
        dram = ctx.enter_context(tc.tile_pool(name="dram", bufs=1, space="DRAM"))

        # ---- loads ------------------------------------------------------
        x_sb = big.tile([P, CT, N], F32R)  # keys ordered [mine | other]
        xm_r = x_mine.rearrange("(ct p) n -> p ct n", p=P)
        for ct in range(CT):
            for q8 in range(4):
                nc.sync.dma_start(
                    out=x_sb[:, ct, q8 * (NH // 4): (q8 + 1) * (NH // 4)],
                    in_=xm_r[:, ct, q8 * (NH // 4): (q8 + 1) * (NH // 4)],
                )
        nc.sync.dma_start(
            out=x_sb[:, :, NH:N], in_=x_other.rearrange("(ct p) n -> p ct n", p=P)
        )
        wq_sb = big.tile([P, CT, 3 * C], BF16)
        nc.sync.dma_start(out=wq_sb, in_=w_qkvT.rearrange("(ct p) o -> p ct o", p=P))
        wo_sb = big.tile([P, CT, C], BF16)
        nc.sync.dma_start(out=wo_sb, in_=w_outT.rearrange("(ct p) o -> p ct o", p=P))
        bnw_sb = small.tile([P, CT, 1], F32)
        nc.sync.dma_start(out=bnw_sb, in_=bn_w)
        bnb_sb = small.tile([P, CT, 1], F32)
        nc.sync.dma_start(out=bnb_sb, in_=bn_b)
        bo_sb = small.tile([P, CT, 1], F32)
        nc.sync.dma_start(out=bo_sb, in_=b_out)

        # preload the exp activation table while DMAs run
        scr = small.tile([1, 2], F32)
        nc.vector.memset(scr[:, 0:1], 0.0)
        nmarg = small.tile([P, 1], F32)
        nc.vector.memset(nmarg, -MARGIN)
        nc.scalar.activation(out=scr[:, 1:2], in_=scr[:, 0:1], func=AF.Exp,
                             bias=nmarg[0:1])

        # ---- BN stats over my (batch, half) slice + AllReduce -----------
        SG = NH // 512
        stat6 = small.tile([P, CT, SG, 6], F32)
        for ct in range(CT):
            xm = x_sb.bitcast(F32)[:, ct, 0:NH].rearrange("p (s f) -> p s f", f=512)
            for s in range(SG):
                nc.vector.bn_stats(out=stat6[:, ct, s, :], in_=xm[:, s, :])
        mv = small.tile([P, CT, 2], F32)
        for ct in range(CT):
            nc.vector.bn_aggr(out=mv[:, ct, :], in_=stat6[:, ct])

        # pack [mean, E[x^2]] = [mean, var + mean^2]
        ccin_sb = small.tile([P, CT, 2], F32)
        nc.vector.tensor_copy(out=ccin_sb[:, :, 0:1], in_=mv[:, :, 0:1])
        msq = small.tile([P, CT, 1], F32)
        nc.vector.tensor_mul(out=msq, in0=mv[:, :, 0:1], in1=mv[:, :, 0:1])
        nc.vector.tensor_add(out=ccin_sb[:, :, 1:2], in0=mv[:, :, 1:2], in1=msq)

        cc_in = dram.tile([C, 2], F32)
        cc_out = dram.tile([NCORES, C, 2], F32, addr_space="Shared")
        nc.sync.dma_start(out=cc_in.rearrange("(ct p) s -> p ct s", p=P), in_=ccin_sb)
        nc.gpsimd.collective_compute(
            "AllGather",
            OP.bypass,
            replica_groups=[list(range(NCORES))],
            ins=[cc_in.opt()],
            outs=[cc_out.opt()],
        )
        g8 = small.tile([P, NCORES, CT, 2], F32)
        nc.sync.dma_start(
            out=g8, in_=cc_out.rearrange("r (ct p) s -> p r ct s", p=P))
        g8f = g8.rearrange("p r ct s -> p (r ct s)")
        g16 = small.tile([P, 16], F32)
        nc.vector.tensor_add(out=g16, in0=g8f[:, 0:16], in1=g8f[:, 16:32])
        g4t = small.tile([P, 8], F32)
        nc.vector.tensor_add(out=g4t, in0=g16[:, 0:8], in1=g16[:, 8:16])
        gstat = small.tile([P, CT, 2], F32)
        nc.vector.tensor_add(out=gstat.rearrange("p ct s -> p (ct s)"),
                             in0=g4t[:, 0:4], in1=g4t[:, 4:8])

        # raw x -> bf16 during the collective wait (engines are idle then);
        # BN scale folds into the weights, shift into the psum-drain copies
        xbr = big.tile([P, CT, N], BF16)
        for ct in range(CT):
            nc.vector.tensor_copy(out=xbr[:, ct, :], in_=x_sb.bitcast(F32)[:, ct, :])

        # zero/ones padding of the fp8 AV stationary tiles (DVE; gpsimd must
        # stay free so the collective isn't queued behind memsets)
        vT_pad = big.tile([P, H, NKC // 2, 2, VP], U8)
        nc.vector.memset(vT_pad[:, :, :, :, D:VP], 0)
        nc.vector.memset(vT_pad[:, :, :, :, D:D + 1].bitcast(E4), 1.0)

        # warm the PE a little during the collective wait
        wt = small.tile([1, P], BF16)
        nc.vector.memset(wt, 1.0)
        for _ in range(24):
            ws = mmpool.tile([P, QS], F32, tag="mm", name="warm")
            nc.tensor.matmul(out=ws[0:1, 0:P], lhsT=wt[0:1, 0:1], rhs=wt,
                             start=True, stop=True)

        # global mean/var -> s = bn_w * rstd, t = bn_b - mean * s
        mean_g = small.tile([P, CT, 1], F32)
        nc.vector.tensor_scalar_mul(out=mean_g, in0=gstat[:, :, 0:1],
                                    scalar1=1.0 / NCORES)
        e2 = small.tile([P, CT, 1], F32)
        nc.vector.tensor_scalar_mul(out=e2, in0=gstat[:, :, 1:2],
                                    scalar1=1.0 / NCORES)
        var_g = small.tile([P, CT, 1], F32)
        nc.vector.tensor_mul(out=var_g, in0=mean_g, in1=mean_g)
        nc.vector.tensor_sub(out=var_g, in0=e2, in1=var_g)
        eps_sb = small.tile([P, 1], F32)
        nc.vector.memset(eps_sb, EPS)
        sd = small.tile([P, CT, 1], F32)
        nc.scalar.activation(out=sd, in_=var_g, func=AF.Sqrt, bias=eps_sb)
        rstd = small.tile([P, CT, 1], F32)
        nc.vector.reciprocal(out=rstd, in_=sd)
        s_sb = small.tile([P, CT, 1], F32)
        nc.vector.tensor_mul(out=s_sb, in0=bnw_sb, in1=rstd)
        t_sb = small.tile([P, CT, 1], F32)
        nc.vector.tensor_mul(out=t_sb, in0=mean_g, in1=s_sb)
        nc.vector.tensor_sub(out=t_sb, in0=bnb_sb, in1=t_sb)

        # b_qkv[o] = sum_c W[c,o] * t_c (with original host-scaled W), then
        # fold s into W rows in place: projections then run on raw bf16 x
        t16 = small.tile([P, CT, 1], BF16)
        nc.vector.tensor_copy(out=t16, in_=t_sb)
        bq_ps = mmpool.tile([1, 2 * C], F32, tag="mm", name="bqps")
        for ct in range(CT):
            nc.tensor.matmul(out=bq_ps, lhsT=t16[:, ct, :],
                             rhs=wq_sb[:, ct, 0:2 * C],
                             start=(ct == 0), stop=(ct == CT - 1))
        bv_ps = mmpool.tile([1, C], F32, tag="mm", name="bvps")
        for ct in range(CT):
            nc.tensor.matmul(out=bv_ps, lhsT=t16[:, ct, :],
                             rhs=wq_sb[:, ct, 2 * C:3 * C],
                             start=(ct == 0), stop=(ct == CT - 1))
        bvrow = small.tile([1, 3 * C], F32)
        nc.vector.tensor_copy(out=bvrow[:, 0:2 * C], in_=bq_ps)
        nc.vector.tensor_copy(out=bvrow[:, 2 * C:3 * C], in_=bv_ps)
        bdr = dram.tile([3 * C, 1], F32)
        nc.sync.dma_start(out=bdr.rearrange("(a f) b -> a (f b)", a=1), in_=bvrow)
        bvec = small.tile([P, 6], F32)
        nc.sync.dma_start(out=bvec,
                          in_=bdr.rearrange("(c p) b -> p (c b)", p=P))
        bvb = small.tile([P, C], F32)
        nc.gpsimd.partition_broadcast(bvb, bvrow[:, 2 * C:3 * C])
        for ct in range(CT):
            nc.vector.tensor_scalar_mul(out=wq_sb[:, ct, :], in0=wq_sb[:, ct, :],
                                        scalar1=s_sb[:, ct])

        # ---- QKV projections (all bf16) ---------------------------------
        # k_nat[0:64, rb, :]   = head 2rb   k-channels, natural key order
        # k_nat[64:128, rb, :] = head 2rb+1
        k_nat = big.tile([P, RB, N], BF16)
        q_pad = big.tile([P, RB * NQS, QS], BF16)  # verbatim q-proj psum copies
        attn_sb = big.tile([P, RB, NH], BF16)
        eb_r = epool  # alias

        # k projection (ACT copies)
        for rb in range(RB):
            for jk in range(N // QS):
                ps = mmpool.tile([P, QS], F32, tag="mm")
                for ct in range(CT):
                    nc.tensor.matmul(
                        out=ps,
                        lhsT=wq_sb[:, ct, C + rb * P: C + (rb + 1) * P],
                        rhs=xbr[:, ct, jk * QS:(jk + 1) * QS],
                        start=(ct == 0), stop=(ct == CT - 1),
                    )
                nc.scalar.activation(out=k_nat[:, rb, jk * QS:(jk + 1) * QS],
                                     in_=ps, func=AF.Identity,
                                     bias=bvec[:, 2 + rb:3 + rb], scale=1.0)

        def qproj(rb, j):
            ps = mmpool.tile([P, QS], F32, tag="mm", name="qps")
            for ct in range(CT):
                nc.tensor.matmul(
                    out=ps,
                    lhsT=wq_sb[:, ct, rb * P:(rb + 1) * P],
                    rhs=xbr[:, ct, j * QS:(j + 1) * QS],
                    start=(ct == 0), stop=(ct == CT - 1),
                )
            nc.vector.tensor_scalar_add(out=q_pad[:, rb * NQS + j, :],
                                        in0=ps, scalar1=bvec[:, rb:rb + 1])

        # q for j=0 first (unblocks attention), then v, then remaining q
        for rb in range(RB):
            qproj(rb, 0)

        # v projection, transposed: psum [key, (h d)] pairs of chunks share
        # one bank; copy+cast straight to fp8 with per-head placement (DVE)
        for pv in range(NKC // 2):
            ps = mmpool.tile([P, 2, C], F32, tag="mm")
            for ko in range(2):
                nb = 2 * pv + ko
                for ct in range(CT):
                    nc.tensor.matmul(
                        out=ps[:, ko, :],
                        lhsT=xbr[:, ct, nb * KC:(nb + 1) * KC],
                        rhs=wq_sb[:, ct, 2 * C:3 * C],
                        start=(ct == 0), stop=(ct == CT - 1),
                    )
            for ko in range(2):
                nc.vector.tensor_tensor(
                    out=vT_pad[:, :, pv, ko, 0:D].bitcast(E4),
                    in0=ps[:, ko, :].rearrange("p (h c) -> p h c", c=D),
                    in1=bvb.rearrange("p (h c) -> p h c", c=D),
                    op=OP.add,
                )

        for j in range(1, NQS):
            for rb in range(RB):
                qproj(rb, j)

        # ---- attention --------------------------------------------------
        out_r = out.rearrange("(rb p) n -> p rb n", p=P)

        def normalize(avp_, rb_, h2_, j_):
            dstage = rpool.tile([1, QS], F32, tag="ds")
            nc.vector.tensor_copy(out=dstage, in_=avp_[D:D + 1, :])
            r_sb = rpool.tile([1, QS], F32, tag="r")
            nc.vector.reciprocal_approx_fast(out=r_sb, in_=dstage)
            rbc = rpool.tile([D, QS], F32, tag="rbc")
            nc.gpsimd.partition_broadcast(rbc, r_sb)
            nc.vector.tensor_tensor(
                out=attn_sb[h2_ * D:(h2_ + 1) * D, rb_, j_ * QS:(j_ + 1) * QS],
                in0=avp_[0:D, :], in1=rbc, op=OP.mult,
            )

        def outproj(j_):
            for rbo in range(RB):
                po = mmpool.tile([P, QS], F32, tag="mm", name="po")
                for ct in range(CT):
                    nc.tensor.matmul(
                        out=po,
                        lhsT=wo_sb[:, ct, rbo * P:(rbo + 1) * P],
                        rhs=attn_sb[:, ct, j_ * QS:(j_ + 1) * QS],
                        start=(ct == 0), stop=(ct == CT - 1),
                    )
                o_t = opool.tile([P, QS], F32, tag="o")
                nc.scalar.activation(out=o_t, in_=po, func=AF.Identity,
                                     bias=bo_sb[:, rbo], scale=1.0)
                nc.sync.dma_start(out=out_r[:, rbo, j_ * QS:(j_ + 1) * QS],
                                  in_=o_t)

        for j in range(NQS):
            for rb in range(RB):
                eb = epool.tile([P, SLOTS, QS], U8, tag="e")
                eb5 = eb.rearrange("pt (p ko h) c -> pt p ko h c", ko=2, h=2)
                avp = [None, None]
                pend = []  # delayed AV emits: (h2, pair, start, stop)

                def flush_pend(upto_g):
                    while pend and pend[0][0] <= upto_g:
                        (_, h2_, p_, st_, sp_) = pend.pop(0)
                        nc.tensor.matmul(
                            out=avp[h2_][0:VP, :],
                            lhsT=vT_pad[:, 2 * rb + h2_, p_].bitcast(E4),
                            rhs=eb5[:, p_, :, h2_, :].bitcast(E4),
                            start=st_, stop=sp_,
                            perf_mode=mybir.MatmulPerfMode.DoubleRow,
                        )
                        if sp_:
                            normalize(avp[h2_], rb, h2_, j)

                avp[0] = mmpool.tile([P, QS], F32, tag="mm", name="avp0")
                avp[1] = mmpool.tile([P, QS], F32, tag="mm", name="avp1")
                for g in range(NGR):
                    gs = min(G, SLOTS - g * G)
                    # AVs whose exp finished >= 2 groups ago sit at the PE
                    # queue head without blocking (their input is long done)
                    flush_pend(g - 2)
                    st = spool.tile([P, G, QS], F32, tag="sp")
                    for u in range(gs):
                        slot = g * G + u
                        c, h2 = slot // 2, slot % 2
                        nc.tensor.matmul(
                            out=st[:, u, :],
                            lhsT=k_nat[h2 * D:(h2 + 1) * D, rb,
                                       c * KC:(c + 1) * KC],
                            rhs=q_pad[h2 * D:(h2 + 1) * D, rb * NQS + j, :],
                            start=True, stop=True,
                            tile_position=(h2 * D, 0),
                        )
                    if g in ACT_GROUPS:
                        nc.scalar.activation(
                            out=eb[:, g * G:g * G + gs, :].bitcast(E4),
                            in_=st[:, 0:gs, :], func=AF.Exp,
                            bias=nmarg, scale=1.0,
                        )
                    else:
                        nc.vector.tensor_scalar(
                            out=eb[:, g * G:g * G + gs, :],
                            in0=st[:, 0:gs, :],
                            scalar1=FE_A, scalar2=FE_B,
                            op0=OP.mult, op1=OP.add,
                        )
                    for u in range(gs):
                        slot = g * G + u
                        if slot >= 2 and (slot - 2) % 4 in (0, 1):
                            p_, h2_ = (slot - 2) // 4, (slot - 2) % 4
                            pend.append((g, h2_, p_, p_ == 0,
                                         p_ == NKC // 2 - 1))
                flush_pend(NGR + 2)
                if dbg is not None and j == 0 and rb == 0:
                    nc.sync.dma_start(out=dbg["eb"], in_=eb)
            outproj(j)
        if dbg is not None:
            nc.sync.dma_start(out=dbg["xb"], in_=xbr)
            nc.sync.dma_start(out=dbg["q"], in_=q_pad)
            nc.sync.dma_start(out=dbg["k"], in_=k_nat)
            nc.sync.dma_start(out=dbg["attn"], in_=attn_sb)
            nc.sync.dma_start(out=dbg["vt"], in_=vT_pad)


def build():
    nc = bacc.Bacc("TRN2", target_bir_lowering=False, debug=False,
                   num_devices=NCORES)
    x_mine = nc.dram_tensor("x_mine", [C, NH], F32R, kind="ExternalInput").ap()
    x_other = nc.dram_tensor("x_other", [C, NH], F32R, kind="ExternalInput").ap()
    w_qkvT = nc.dram_tensor("w_qkvT", [C, 3 * C], BF16, kind="ExternalInput").ap()
    w_outT = nc.dram_tensor("w_outT", [C, C], BF16, kind="ExternalInput").ap()
    bn_w = nc.dram_tensor("bn_w", [P, CT, 1], F32, kind="ExternalInput").ap()
    bn_b = nc.dram_tensor("bn_b", [P, CT, 1], F32, kind="ExternalInput").ap()
    b_out = nc.dram_tensor("b_out", [P, CT, 1], F32, kind="ExternalInput").ap()
    out = nc.dram_tensor("out", [C, NH], F32, kind="ExternalOutput").ap()
    dbg = None
    if DEBUG:
        dbg = {
            "eb": nc.dram_tensor("dbg_eb", [P, SLOTS, QS], U8, kind="ExternalOutput").ap(),
            "xb": nc.dram_tensor("dbg_xb", [P, CT, N], BF16, kind="ExternalOutput").ap(),
            "q": nc.dram_tensor("dbg_q", [P, RB * NQS, QS], BF16, kind="ExternalOutput").ap(),
            "k": nc.dram_tensor("dbg_k", [P, RB, N], BF16, kind="ExternalOutput").ap(),
            "attn": nc.dram_tensor("dbg_attn", [P, RB, NH], BF16, kind="ExternalOutput").ap(),
            "vt": nc.dram_tensor("dbg_vt", [P, H, NKC // 2, 2, VP], U8, kind="ExternalOutput").ap(),
        }
    with tile.TileContext(nc) as tc:
        _body(tc, x_mine, x_other, w_qkvT, w_outT, bn_w, bn_b, b_out, out,
              dbg)
    nc.compile()
    return nc


_nc_cache = None


def make_in_maps(x, bn_weight, bn_bias, w_qkv, w_out, b_out):
    x = np.ascontiguousarray(np.asarray(x, dtype=np.float32))
    wq = np.asarray(w_qkv, dtype=np.float32).copy()
    wq[0:C, :] *= SCALE  # fold q scaling d^-0.5 into the q rows
    wqT = np.ascontiguousarray(wq.T).astype(ml_dtypes.bfloat16)
    woT = np.ascontiguousarray(np.asarray(w_out, dtype=np.float32).T).astype(
        ml_dtypes.bfloat16)

    def vec_layout(v):
        v = np.asarray(v, dtype=np.float32)
        return np.ascontiguousarray(v.reshape(CT, P).T.reshape(P, CT, 1))

    bnw = vec_layout(bn_weight)
    bnb = vec_layout(bn_bias)
    bo = vec_layout(b_out)
    in_maps = []
    for core in range(NCORES):
        bi, half = divmod(core, 2)
        mine = np.ascontiguousarray(x[bi][:, half * NH:(half + 1) * NH])
        other = np.ascontiguousarray(x[bi][:, (1 - half) * NH:(2 - half) * NH])
        in_maps.append({
            "x_mine": mine, "x_other": other, "w_qkvT": wqT, "w_outT": woT,
            "bn_w": bnw, "bn_b": bnb, "b_out": bo,
        })
    return in_maps


def assemble(results):
    outp = np.empty((B, C, N), np.float32)
    for core in range(NCORES):
        bi, half = divmod(core, 2)
        outp[bi][:, half * NH:(half + 1) * NH] = results[core]["out"]
    return outp


def kernel(x, bn_weight, bn_bias, w_qkv, w_out, b_out):
    global _nc_cache
    if _nc_cache is None:
        _nc_cache = build()
    in_maps = make_in_maps(x, bn_weight, bn_bias, w_qkv, w_out, b_out)
    res = run_bass_kernel_spmd(_nc_cache, in_maps, list(range(NCORES)))
    return assemble(res.results)
